# revision 29
# baseline (speedup 1.0000x reference)
"""EnhancedTemporalAttention Trainium2 kernel (v3, fp8 DoubleRow).

Full module: GroupNorm(32) -> QKV 1x1conv -> 8-head attention (softmax) ->
out 1x1conv + bias -> +residual, on x [4, 512, 2048] fp32.

Sharding: 8 cores = (batch b = core//2) x (head-half hg = core%2).  Each
core computes GroupNorm stats + its 4 heads over the full sequence and a
partial out-projection (contraction over its 256 channels); the host sums
the two partials per batch and adds residual + b_out in fp32.

All heavy matmuls run in fp8 with the DoubleRow perf mode (2 contraction
rows per PE cell, 0.5 cycles/row):
  - QKV: x and (GroupNorm-folded) weights in e4m3, channel tiles paired as
    DR slots.
  - Scores: q/k stored as [h*32+d%32, d//32-slot, n] e4m3 so each head's
    64-dim contraction is 32 partitions x 2 slots.
  - AV: e^T stationary with CONSECUTIVE KEY BLOCKS as the two DR slots
    (256 keys per matmul); v carries a 1/32 ones column so softmax
    denominators ride the same PSUM tile (attn out is scaled x32, undone
    on the host).
  - Out-projection: attnout (e4m3) with the two head-pairs as DR slots.

exp splits three ways: ACT computes exact exp (scale 1/8, bias -SHIFT to
keep e4m3 finite) straight to e4m3; DVE and GPSIMD run an int8 Schraudolph
(i8 = s*A + B bitcast e5m2).  The softmax denominator uses fp32 PSUM, so
the shift cancels exactly.

PSUM: 3-slot ring of [128,1024] score tiles + one [128,8,128] AV
accumulator = 16KB exactly; transposes / out-proj / GN matvecs borrow
ring slots via half-slot views.
"""
import sys

sys.path.insert(0, "/opt/trn_rl_repo")

import numpy as np
import ml_dtypes

import concourse.bacc as bacc
import concourse.bass as bass
import concourse.tile as tile
from concourse import mybir
from concourse.bass_utils import run_bass_kernel_spmd

F32 = mybir.dt.float32
F32R = mybir.dt.float32r
BF16 = mybir.dt.bfloat16
I16 = mybir.dt.int16
I8 = mybir.dt.int8
E4 = mybir.dt.float8e4
E5 = mybir.dt.float8e5

B = 4
C = 512
N = 2048
H = 8
HL = 4             # local heads per core
D = 64
G = 32             # groupnorm groups
CPG = C // G       # 16 channels per group
UN = 4             # input-channel units of 128 (u = t*2 + slot)
EPS = 1e-4
SCALE = D ** -0.5
NKB = N // 128     # 16 key blocks
NJP = NKB // 2     # 8 key-block pairs (DR slots)
QC = 4             # query chunks of 512
AVS = 32.0         # attnout scale (ones col = 1/AVS, host divides)
SHIFT = 2.2        # exp(l - SHIFT): keeps e4m3 finite; cancels in softmax
AF = mybir.ActivationFunctionType
ALU = mybir.AluOpType
PM = mybir.MatmulPerfMode

# Schraudolph exp into e5m2 bits: i8 = s*A5 + B5, bitcast -> fp8e5
A5 = (4.0 / np.log(2.0)) * SCALE
B5 = 60.25 - (4.0 / np.log(2.0)) * SHIFT

# Engine choice: GPSIMD cannot touch PSUM, so exp + every PSUM-reading
# copy runs on ACT or DVE.  A greedy build-time allocator assigns each
# task to the engine with the lower projected busy time; ACT exp units
# get exact exp -> e4m3, DVE units the e5m2 Schraudolph.


def _build():
    nc = bacc.Bacc("TRN2", target_bir_lowering=False, debug=False)
    x_in = nc.dram_tensor("x8", [128, UN, N], E4, kind="ExternalInput").ap()
    w8_in = nc.dram_tensor("w8", [128, UN, 768], E4,
                           kind="ExternalInput").ap()
    wout_in = nc.dram_tensor("wout8", [128, 2, C], E4,
                             kind="ExternalInput").ap()
    qkvb_in = nc.dram_tensor("qkvb", [128, 6], F32,
                             kind="ExternalInput").ap()
    id_in = nc.dram_tensor("ident", [128, 128], BF16,
                           kind="ExternalInput").ap()
    y_out = nc.dram_tensor("y", [C, N], F32, kind="ExternalOutput").ap()

    from contextlib import ExitStack
    with tile.TileContext(nc) as tc, ExitStack() as ctx:
        persist = ctx.enter_context(tc.tile_pool(name="persist", bufs=1))
        gn = ctx.enter_context(tc.tile_pool(name="gn", bufs=1))
        pspool = ctx.enter_context(tc.tile_pool(name="ps", bufs=1,
                                                space="PSUM"))
        expp = ctx.enter_context(tc.tile_pool(name="expp", bufs=1))
        drp = ctx.enter_context(tc.tile_pool(name="drp", bufs=1))

        # ---- persistent tiles ----
        x8 = persist.tile([128, UN, N], E4, tag="x8", name="x8")
        w8 = persist.tile([128, UN, 768], E4, tag="w8", name="w8")
        w8out = persist.tile([128, 2, C], E4, tag="w8out", name="w8out")
        q8 = persist.tile([128, 2, N], E4, tag="q8", name="q8")
        k8 = persist.tile([128, 2, N], E4, tag="k8", name="k8")
        v2 = [persist.tile([128, 2, HL, 66], E4, tag=f"v2_{jp}",
                           name=f"v2_{jp}") for jp in range(NJP)]
        ident = persist.tile([128, 128], BF16, tag="ident", name="ident")

        # PSUM: S ring 3x[128,1024] (4KB each) + av [128,8,128] (4KB)
        def new_S():
            return pspool.tile([128, 1024], F32, tag="S", name="S", bufs=3)

        def new_ops():
            return new_S()[:, 0:512]

        av = pspool.tile([128, 8, 128], F32, tag="av", name="av", bufs=1)

        # ---- input loads ----
        # GroupNorm statistics are folded into w8/qkvb on the host, so the
        # device goes straight from DMA to the projections.
        qkvb = gn.tile([128, 6], F32, tag="qkvb")
        shift_t = gn.tile([128, 1], F32, tag="shift_t")
        nc.gpsimd.memset(shift_t, -SHIFT)
        # x8 rides the (serialized) HWDGE; weights + misc go through the
        # software DGE so they do not queue behind it.
        for u in range(UN):
            nc.sync.dma_start(out=x8[:, u, :], in_=x_in[:, u, :])
        for u in range(UN):
            nc.gpsimd.dma_start(out=w8[:, u, :], in_=w8_in[:, u, :])
        nc.gpsimd.dma_start(out=qkvb, in_=qkvb_in)
        nc.gpsimd.dma_start(out=w8out.rearrange("p m c -> p (m c)"),
                            in_=wout_in.rearrange("p m c -> p (m c)"))
        nc.gpsimd.dma_start(out=ident, in_=id_in)
        # preload the Exp table while DMAs land
        warm = gn.tile([128, 1], F32, tag="warm")
        nc.scalar.activation(out=warm, in_=shift_t, func=AF.Exp)

        # ---- projections ----
        # greedy ACT/DVE load balancing (ns estimates from the cost model)
        eng_busy = {"act": 0.0, "dve": 0.0}
        unit_eng = {}

        def pick(act_cost, dve_cost):
            # DVE absorbs the S-slot refill waits; bias work toward ACT
            if eng_busy["act"] + act_cost <= (eng_busy["dve"] + dve_cost) * 1.06:
                eng_busy["act"] += act_cost
                return "act"
            eng_busy["dve"] += dve_cost
            return "dve"

        def kq_proj(which, s, ncx, engi):
            """q/k for d-half slot s, query chunk ncx -> q8/k8[:, s, cols]."""
            ps = new_ops()
            col0 = (s if which == "q" else 2 + s) * 128
            for t in range(2):
                nc.tensor.matmul(
                    ps, lhsT=w8[:, 2 * t:2 * t + 2, col0:col0 + 128],
                    rhs=x8[:, 2 * t:2 * t + 2, ncx * 512:(ncx + 1) * 512],
                    start=(t == 0), stop=(t == 1), perf_mode=PM.DoubleRow)
            dst = (q8 if which == "q" else k8)[:, s, ncx * 512:(ncx + 1) * 512]
            boff = (0 if which == "q" else 2) + s
            if pick(612, 658) == "act":
                nc.scalar.activation(out=dst, in_=ps, func=AF.Identity,
                                     bias=qkvb[:, boff:boff + 1])
            else:
                nc.vector.tensor_scalar(out=dst, in0=ps,
                                        scalar1=qkvb[:, boff:boff + 1],
                                        scalar2=None, op0=ALU.add)

        def v_proj(jp, engi):
            """v for key blocks 2jp,2jp+1 -> v2[jp][:, :, :, 0:64]."""
            ps = new_ops()
            for i in range(2):
                nb = 2 * jp + i
                for t in range(2):
                    nc.tensor.matmul(
                        ps[:, i * 256:(i + 1) * 256],
                        lhsT=x8[:, 2 * t:2 * t + 2, nb * 128:(nb + 1) * 128],
                        rhs=w8[:, 2 * t:2 * t + 2, 512:768],
                        start=(i == 0 and t == 0), stop=(t == 1),
                        perf_mode=PM.DoubleRow, skip_group_check=True)
            src = ps.rearrange("p (i h d) -> p i h d", i=2, h=HL)
            dst = v2[jp][:, :, :, 0:64]
            if pick(612, 658) == "act":
                nc.scalar.activation(out=dst, in_=src, func=AF.Copy)
            else:
                nc.vector.tensor_copy(dst, src)
            nc.gpsimd.memset(v2[jp][:, :, :, 64:65], 1.0 / AVS)

        for s in range(2):
            for ncx in range(4):
                kq_proj("k", s, ncx, 0)
        # q chunk 0 now so the attention stream can start
        kq_proj("q", 0, 0, 0)
        kq_proj("q", 1, 0, 1)

        # ---- attention stream ----
        # eT2[(qc, jp)]: [128 keys, 2 j-slots, 2048 (4h x 512q)] int8 ring
        eT2 = {}

        def get_eT2(qc, jp):
            if (qc, jp) not in eT2:
                eT2[(qc, jp)] = expp.tile([128, 2, N], I8, tag="eT2",
                                          name="eT2", bufs=14)
            return eT2[(qc, jp)]

        def emit_scores_exp(qc, j, hp):
            s = new_S()
            for hh in range(2):
                h = hp * 2 + hh
                nc.tensor.matmul(
                    s[:, hh * 512:(hh + 1) * 512],
                    lhsT=k8[h * 32:(h + 1) * 32, :, j * 128:(j + 1) * 128],
                    rhs=q8[h * 32:(h + 1) * 32, :, qc * 512:(qc + 1) * 512],
                    start=True, stop=True, perf_mode=PM.DoubleRow,
                    tile_position=(h * 32, 0), skip_group_check=True)
            et = get_eT2(qc, j // 2)
            dst = et[:, j % 2, hp * 1024:(hp + 1) * 1024]
            key = (qc, j // 2, hp)
            if key not in unit_eng:
                unit_eng[key] = pick(2 * 1038, 2 * 1192)
            eng = unit_eng[key]
            if eng == "act":
                nc.scalar.activation(out=dst.bitcast(E4), in_=s,
                                     func=AF.Exp, scale=SCALE, bias=shift_t)
            else:
                nc.vector.tensor_scalar(out=dst, in0=s, scalar1=A5,
                                        scalar2=B5, op0=ALU.mult,
                                        op1=ALU.add)

        def emit_av(qc, m, jp):
            """8 DR matmuls: 256 keys (j-slot pair) x [128q, 65]."""
            et = get_eT2(qc, jp)
            dt = E4 if unit_eng[(qc, jp, m)] == "act" else E5
            eb = et.bitcast(dt)
            for qb in range(4):
                for hh in range(2):
                    c0 = m * 1024 + hh * 512 + qb * 128
                    sub = qb * 2 + hh
                    # start=True only for the first sub of each 2KB PSUM
                    # bank: the whole-bank pending-zero mark fresh-writes
                    # the other subs' first accumulation.
                    nc.tensor.matmul(
                        av[:, sub, 0:65],
                        lhsT=eb[:, :, c0:c0 + 128],
                        rhs=v2[jp][:, :, 2 * m + hh, 0:65],
                        start=(jp == 0 and sub % 4 == 0),
                        stop=(jp == NJP - 1),
                        perf_mode=PM.DoubleRow, skip_group_check=True)
            if jp == NJP - 1:
                drain_a(qc, m)

        avn_pend = {}

        def drain_a(qc, m):
            """rden + normalize av -> avn bf16 [128, 4qb, 2hh*64d]."""
            rden = drp.tile([128, 8, 1], F32, tag="rden", name="rden",
                            bufs=3)
            eng_busy["dve"] += 660.0
            nc.vector.reciprocal(rden, av[:, :, 64:65])
            avn = drp.tile([128, 4, 128], BF16, tag="avn", name="avn",
                           bufs=3)
            avv = avn.rearrange("p qb c -> p (qb c)").rearrange(
                "p (qb h d) -> p qb h d", qb=4, h=2)
            s1 = rden.ap[1][0]
            rb = bass.AP(tensor=rden.tensor, offset=rden.offset,
                         ap=[rden.ap[0], [s1 * 2, 4], [s1, 2], [0, 64]])
            nc.vector.tensor_tensor(
                out=avv,
                in0=av[:, :, 0:64].rearrange("p (qb h) d -> p qb h d", qb=4),
                in1=rb, op=ALU.mult)
            avn_pend[(qc, m)] = avn

        op_tiles = {}

        def drain_b(qc, m, engi):
            """4 transposes + one bias-add/quantize -> op[:, m, :]."""
            avn = avn_pend[(qc, m)]
            if qc not in op_tiles:
                op_tiles[qc] = drp.tile([128, 2, 512], E4, tag="op",
                                        name="op", bufs=3)
            tps_flat = new_ops().bitcast(BF16)[:, 0:512]
            tps = tps_flat.rearrange("p (qb q) -> p qb q", qb=4)
            for qb in range(4):
                nc.tensor.transpose(tps[:, qb, :], avn[:, qb, :], ident)
            if pick(612, 658) == "act":
                nc.scalar.activation(out=op_tiles[qc][:, m, :], in_=tps_flat,
                                     func=AF.Identity,
                                     bias=qkvb[:, 4 + m:5 + m])
            else:
                nc.vector.tensor_scalar(out=op_tiles[qc][:, m, :],
                                        in0=tps_flat,
                                        scalar1=qkvb[:, 4 + m:5 + m],
                                        scalar2=None, op0=ALU.add)

        def emit_outproj(qc, m2, engi):
            ps = new_ops()
            nc.tensor.matmul(ps, lhsT=w8out[:, :, m2 * 128:(m2 + 1) * 128],
                             rhs=op_tiles[qc], start=True, stop=True,
                             perf_mode=PM.DoubleRow, skip_group_check=True)
            yt = drp.tile([128, 512], F32, tag="yt", name="yt", bufs=6)
            if pick(612, 658) == "act":
                nc.scalar.activation(out=yt, in_=ps, func=AF.Copy)
            else:
                nc.vector.tensor_copy(yt, ps)
            nc.sync.dma_start(
                out=y_out[m2 * 128:(m2 + 1) * 128,
                          qc * 512:(qc + 1) * 512],
                in_=yt)

        # Event-driven emission: per step (qc, j) emit scores+exp, then any
        # due deferred work (v/q proj early, AV batches, drains, outproj).
        events = {}     # step -> list of callables

        def at(step, fn):
            events.setdefault(step, []).append(fn)

        # q for query chunks 1..3 early in the qc0 stream
        for i, (s, ncx) in enumerate([(s, ncx) for ncx in range(1, 4)
                                      for s in range(2)]):
            at(i, (lambda s=s, ncx=ncx: kq_proj("q", s, ncx, 0)))
        # v projections: pair jp at step 2jp+1
        for jp in range(NJP):
            at(2 * jp + 1, (lambda jp=jp: v_proj(jp, jp)))

        rr = [0]

        def nrr():
            rr[0] += 1
            return rr[0]

        total_steps = QC * NKB
        for qc in range(QC):
            base = qc * NKB
            # m0 AV batches chase the exp stream with a wide margin so the
            # av WAR wait (on the previous pair's drain_a) never blocks the
            # PE wait queue while scores still need to flow.
            for jp in range(NJP):
                at(base + max(2 * jp + 3, 9), (lambda qc=qc, jp=jp:
                                               emit_av(qc, m=0, jp=jp)))
            # m0 drain_b 2 steps after drain_a (inside the jp7 AV)
            at(base + NKB + 3, (lambda qc=qc: drain_b(qc, 0, nrr())))
            # m1 AV burst after m0 drained: 2 jp per step
            for i in range(4):
                def m1burst(qc=qc, i=i):
                    emit_av(qc, 1, 2 * i)
                    emit_av(qc, 1, 2 * i + 1)
                at(base + NKB + 4 + i, m1burst)
            at(base + NKB + 9, (lambda qc=qc: drain_b(qc, 1, nrr())))
            for m2 in range(4):
                at(base + NKB + 13 + m2 // 2,
                   (lambda qc=qc, m2=m2: emit_outproj(qc, m2, nrr())))

        max_step = max(events) + 2
        for step in range(max_step):
            if step < total_steps:
                qc, j = step // NKB, step % NKB
                emit_scores_exp(qc, j, 0)
                emit_scores_exp(qc, j, 1)
            for fn in events.get(step - 1, ()):
                fn()

    nc.compile()
    return nc


_NC = None


def _get_nc():
    global _NC
    if _NC is None:
        _NC = _build()
    return _NC


def kernel(x, gn_gamma, gn_beta, w_qkv, w_out, b_out, trace=False):
    E4N = ml_dtypes.float8_e4m3
    x = np.asarray(x, dtype=np.float32)
    w_qkv = np.asarray(w_qkv, np.float32)
    w_out = np.asarray(w_out, np.float32)
    gamma = np.asarray(gn_gamma, np.float32).reshape(C)
    beta = np.asarray(gn_beta, np.float32).reshape(C)
    ident = np.eye(128, dtype=np.float32).astype(ml_dtypes.bfloat16)

    nc = _get_nc()
    # per-batch: GN stats + x8 layout (shared by the two head-half cores)
    x8s, scs, bcs = [], [], []
    for b in range(B):
        xb = x[b]
        xg = xb.reshape(G, CPG * N)
        mean = xg.mean(-1).repeat(CPG)
        var = xg.var(-1).repeat(CPG)
        sc = gamma / np.sqrt(var + EPS)
        bcs.append(beta - mean * sc)
        scs.append(sc)
        x8s.append(np.ascontiguousarray(
            xb.reshape(UN, 128, N).transpose(1, 0, 2)).astype(E4N))

    in_maps = []
    for core in range(8):
        b, hg = core // 2, core % 2
        hgr = slice(hg * 256, (hg + 1) * 256)
        # rows ordered [q_s0 | q_s1 | k_s0 | k_s1 | v] for this head-half
        wq = w_qkv[0:C][hgr].reshape(HL, 2, 32, C)      # [h, s, dd, c]
        wk = w_qkv[C:2 * C][hgr].reshape(HL, 2, 32, C)
        wv = w_qkv[2 * C:3 * C][hgr].reshape(256, C)
        rows = np.concatenate([
            wq[:, 0].reshape(128, C), wq[:, 1].reshape(128, C),
            wk[:, 0].reshape(128, C), wk[:, 1].reshape(128, C),
            wv], axis=0)                                 # [768, C]
        # fold GN scale into the weights; bias via matvec (fp32, exact)
        wf = rows * scs[b][None, :]
        w8 = np.ascontiguousarray(
            wf.T.reshape(UN, 128, 768).transpose(1, 0, 2)).astype(E4N)
        qkvb = (rows @ bcs[b]).reshape(6, 128).T.copy()  # [128, 6]
        qkvb[:, 4:6] *= AVS                              # v bias rides x32
        wo = w_out[:, hgr]                               # [C, 256]
        w8o = np.ascontiguousarray(
            wo.T.reshape(2, 128, C).transpose(1, 0, 2)).astype(E4N)
        in_maps.append({
            "x8": x8s[b],
            "w8": w8,
            "wout8": w8o,
            "qkvb": np.ascontiguousarray(qkvb, dtype=np.float32),
            "ident": ident,
        })
    res = run_bass_kernel_spmd(nc, in_maps, core_ids=list(range(8)),
                               trace=trace)
    y = np.empty((B, C, N), dtype=np.float32)
    bo = np.asarray(b_out, np.float32).reshape(C, 1)
    for b in range(B):
        y[b] = ((res.results[2 * b]["y"] + res.results[2 * b + 1]["y"])
                * np.float32(1.0 / AVS) + x[b] + bo)
    if trace:
        kernel.last_results = res
    return y
